# revision 51
# baseline (speedup 1.0000x reference)
"""Enframe kernel for Trainium2 (Bass/Tile), 8-core data parallel.

Problem: input (16, 480000) f32, frame_length=2048, hop=512.
  out[b, w, f] = input[b, w + 512*f],  f in [0, 934), w in [0, 2048).

Key identity: write w = 512*h + l (h in [0,4), l in [0,512)). Then
  out[b, 512*h + l, f] = input[b, 512*(f + h) + l] = in3[b, f + h, l]
where in3 = input[:, :937*512].reshape(B, 937, 512). So the whole op is ONE
(937, 512) -> (512, 937) transpose per clip; the four h-blocks of the output
are shifted overlapping windows T[:, h : h+934] of that transpose.

Shipped default "v1" (~60-70 us/iter measured, DMA-bound; zero crashes over
hundreds of HW executions):
  - load in3 rows as SBUF A[p = g%128, g//128, 512] (contiguous 2 KB rows)
  - 32 TensorE 128x128 transposes per clip (f32 via identity) into PSUM,
    DVE-copy to SBUF T[p = l%128, l//128, g]
  - 4 stores per clip: DRAM rows (c p) <- T[:, :, h:h+934] via permuted DRAM
    AP; every DMA descriptor is a contiguous 3736 B run.

The "v8*" variants reach ~57-60 us (the measured pure-DMA ceiling) via an
interleaved partition mapping (out row l = 4q + j on partition q, per-h
[128, 4, 934] T tiles -> 14944 B store descriptors), but that family showed
sporadic NRT_EXEC_UNIT_UNRECOVERABLE crashes (4 across ~30 fresh processes,
on both ACT- and DVE-permute versions), so it is not the default. The v1
family ran crash-free for hundreds of executions all session. Known-fatal
on this HW: strided-free-dim f32 lhsT in a transpose matmul (crashes the
NC deterministically).
"""

import numpy as np

N_CORES = 8
BATCH = 16
B = BATCH // N_CORES  # clips per core
S = 480000
FRAME = 2048
HOP = 512
F = (S - FRAME) // HOP + 1  # 934
G = FRAME // HOP + F - 1  # 937 distinct 512-sample rows used
G_FULL = G // 128  # 7 full partition chunks
G_TAIL = G - 128 * G_FULL  # 41
H = FRAME // HOP  # 4 output row-blocks of 512

_CACHE: dict = {}


_VARIANTS = {
    # store_mode: "merged" (4 stores/clip, 1.9 MB, p-major enumeration) or
    #             "per_c" (16 stores/clip, 478 KB, sequential DRAM)
    # split_io: cut loads/stores at the psum-half boundary for earlier starts
    "v1": dict(store_mode="merged", split_io=False, bufs=2, psum_bufs=4),
    "v1p": dict(store_mode="merged", split_io=False, bufs=2, psum_bufs=8),
    # split only the loads (not stores): earlier transpose start, same stores
    "v1L": dict(store_mode="merged", split_io=False, split_loads=True, bufs=2, psum_bufs=4),
    "v2": dict(store_mode="merged", split_io=True, bufs=2, psum_bufs=8),
    "v3": dict(store_mode="per_c", split_io=False, bufs=2, psum_bufs=4),
    "v4": dict(store_mode="merged", split_io=False, bufs=3, psum_bufs=8),
    "v5": dict(store_mode="per_c", split_io=False, bufs=3, psum_bufs=8),
    # ring balance: n of the 8 stores go to the ACT (scalar) ring alongside
    # the loads, to even out bytes between the two HWDGE rings
    "v6": dict(
        store_mode="merged", split_io=False, bufs=2, psum_bufs=4, act_stores=3
    ),
    "v7": dict(
        store_mode="merged", split_io=False, bufs=2, psum_bufs=4, act_stores=2
    ),
    # timing-only: same DMAs, no transpose/copies — measures the pure DMA
    # ceiling of this access pattern (output is garbage)
    "dma": dict(
        store_mode="merged", split_io=False, bufs=2, psum_bufs=4, dma_only=True
    ),
    # dma-only with only half the stores: separates bytes-bound from
    # overhead-bound
    "dma2": dict(
        store_mode="merged",
        split_io=False,
        bufs=2,
        psum_bufs=4,
        dma_only=True,
        store_hs=(0, 1),
    ),
    # dma-only, same bytes but idealized stores: 14992 B descriptors into
    # fully linear DRAM — probes whether descriptor size lifts write BW
    "dma3": dict(
        store_mode="linear", split_io=False, bufs=2, psum_bufs=4, dma_only=True
    ),
    # interleaved partition mapping: output row l = 4q + j lives on partition
    # q, T tiles are per-h [128, 4, 934] so (j, f) merge into one contiguous
    # 3736-element run -> real 14944 B store descriptors
    # final: interleaved partition mapping with contiguous lhsT via ACT
    # pre-permute. NOTE: adding act_stores or split_io here caused
    # NRT_EXEC_UNIT_UNRECOVERABLE crashes (as "v9") — do not re-add.
    "v8": dict(store_mode="interleaved", split_io=False, bufs=2, psum_bufs=4),
    "v8p": dict(store_mode="interleaved", split_io=False, bufs=2, psum_bufs=8),
    # like v8p but the column pre-permute runs on DVE instead of ACT — the
    # ACT-copy version crashed sporadically (NRT_EXEC_UNIT_UNRECOVERABLE)
    "v8d": dict(
        store_mode="interleaved",
        split_io=False,
        bufs=2,
        psum_bufs=8,
        dve_permute=True,
    ),
}


def _build_program(reps: int, variant: str = "v1L"):
    from concourse import bass, masks, mybir
    from concourse.tile import TileContext

    cfg = _VARIANTS[variant]
    split_io = cfg["split_io"]
    store_mode = cfg["store_mode"]
    bufs = cfg["bufs"]
    psum_bufs = cfg["psum_bufs"]
    act_stores = cfg.get("act_stores", 0)
    # spread the ACT-ring stores evenly over the 8 (b, h) store slots
    act_slots = set()
    if act_stores:
        stride = (B * H) / act_stores
        act_slots = {int(i * stride + stride / 2) for i in range(act_stores)}

    F32 = mybir.dt.float32
    nc = bass.Bass()
    inp = nc.declare_dram_parameter("input", [B, S], F32, isOutput=False)
    outp = nc.declare_dram_parameter("out", [B, FRAME, F], F32, isOutput=True)

    with TileContext(nc) as tc:
        with (
            tc.tile_pool(name="ident_pool", bufs=1) as ipool,
            tc.tile_pool(name="a_pool", bufs=bufs) as apool,
            tc.tile_pool(name="t_pool", bufs=bufs) as tpool,
            tc.tile_pool(name="psum_pool", bufs=psum_bufs, space="PSUM") as ppool,
        ):
            ident = ipool.tile([128, 128], F32)
            masks.make_identity(nc, ident[:])

            for _rep in range(reps):
                # loads for both clips upfront (own HWDGE ring via nc.scalar):
                # split at the h8=4 boundary so half-0 transposes start after
                # the first MB.
                a_ts = []
                for b in range(B):
                    a_t = apool.tile([128, G_FULL + 1, HOP], F32, tag="a")
                    a_ts.append(a_t)
                    # rows g = h8*128 + p hold samples 512g .. 512g+512
                    if split_io or cfg.get("split_loads"):
                        nc.scalar.dma_start(
                            out=a_t[:, 0:4, :],
                            in_=inp[b, 0 : 128 * 4 * HOP].rearrange(
                                "(h p c) -> p h c", h=4, p=128, c=HOP
                            ),
                        )
                        nc.scalar.dma_start(
                            out=a_t[:, 4:G_FULL, :],
                            in_=inp[
                                b, 128 * 4 * HOP : 128 * G_FULL * HOP
                            ].rearrange(
                                "(h p c) -> p h c", h=G_FULL - 4, p=128, c=HOP
                            ),
                        )
                    else:
                        nc.scalar.dma_start(
                            out=a_t[:, 0:G_FULL, :],
                            in_=inp[b, 0 : 128 * G_FULL * HOP].rearrange(
                                "(h p c) -> p h c", h=G_FULL, p=128, c=HOP
                            ),
                        )
                    # tail: last 41 rows
                    nc.scalar.dma_start(
                        out=a_t[0:G_TAIL, G_FULL, :],
                        in_=inp[b, 128 * G_FULL * HOP : G * HOP].rearrange(
                            "(p c) -> p c", p=G_TAIL, c=HOP
                        ),
                    )

                for b in range(B):
                    a_t = a_ts[b]
                    if store_mode == "interleaved":
                        # T2h[q, j, f] = out[b, 512h + 4q + j, f]; per-h tiles
                        # of exactly [128, 4, 934] make (j, f) contiguous per
                        # partition -> 14944 B store descriptors.
                        #
                        # A strided-free-dim f32 lhsT crashes the NC
                        # (NRT_EXEC_UNIT_UNRECOVERABLE, probed in isolation),
                        # so pre-permute columns on ACT: a_perm[p, h8, j, q] =
                        # a_t[p, h8, 4q + j]; every matmul then reads a
                        # contiguous 128-column slice.
                        a_perm = apool.tile(
                            [128, G_FULL + 1, 4, 128], F32, tag="a_perm"
                        )
                        perm_copy = (
                            nc.vector.tensor_copy
                            if cfg.get("dve_permute")
                            else nc.scalar.copy
                        )
                        perm_copy(
                            out=a_perm[:, 0:G_FULL, :, :],
                            in_=a_t[:, 0:G_FULL, :].rearrange(
                                "p h (q j) -> p h j q", q=128, j=4
                            ),
                        )
                        perm_copy(
                            out=a_perm[0:G_TAIL, G_FULL, :, :],
                            in_=a_t[0:G_TAIL, G_FULL, :].rearrange(
                                "p (q j) -> p j q", q=128, j=4
                            ),
                        )
                        t2 = [
                            tpool.tile(
                                [128, 4, F], F32, tag=f"t2_{h}", name=f"t2_{h}"
                            )
                            for h in range(H)
                        ]
                        for j in range(4):
                            for half in range(2):
                                ps = ppool.tile([128, 512], F32, tag="ps")
                                glen = 512 if half == 0 else G - 512  # 425
                                for k in range(4):
                                    h8 = 4 * half + k
                                    rows = 128 if h8 < G_FULL else G_TAIL
                                    nc.tensor.transpose(
                                        out=ps[:, 128 * k : 128 * k + rows],
                                        in_=a_perm[0:rows, h8, j, :],
                                        identity=ident[0:rows, 0:rows],
                                    )
                                # ps[q, col] = T row (4q+j), g = 512*half+col
                                for h in range(H):
                                    if half == 0:
                                        # f in [0, 512-h) <- g = h + f
                                        nc.vector.tensor_copy(
                                            out=t2[h][:, j, 0 : 512 - h],
                                            in_=ps[:, h:512],
                                        )
                                    else:
                                        # f in [512-h, ...) <- g = h + f
                                        ln = min(422 + h, glen)
                                        nc.vector.tensor_copy(
                                            out=t2[h][:, j, 512 - h : 512 - h + ln],
                                            in_=ps[:, 0:ln],
                                        )
                        for h in range(H):
                            eng = (
                                nc.scalar
                                if (b * H + h) in act_slots
                                else nc.sync
                            )
                            eng.dma_start(
                                out=outp[b, 512 * h : 512 * (h + 1), :].rearrange(
                                    "(q j) f -> q (j f)", q=128, j=4
                                ),
                                in_=t2[h][:, :, :].rearrange("p j f -> p (j f)"),
                            )
                        continue
                    t_t = tpool.tile([128, 4, G], F32, tag="t")
                    if cfg.get("dma_only"):
                        # give t_t a writer so Tile allocates it
                        nc.vector.memset(t_t[:, 0, 0:1], 0.0)
                    for c in range(4):
                        if cfg.get("dma_only"):
                            break
                        for half in range(2):
                            ps = ppool.tile([128, 512], F32, tag="ps")
                            glen = 512 if half == 0 else G - 512  # 425
                            for k in range(4):
                                h8 = 4 * half + k
                                rows = 128 if h8 < G_FULL else G_TAIL
                                nc.tensor.transpose(
                                    out=ps[:, 128 * k : 128 * k + rows],
                                    in_=a_t[0:rows, h8, 128 * c : 128 * (c + 1)],
                                    identity=ident[0:rows, 0:rows],
                                )
                            nc.vector.tensor_copy(
                                out=t_t[:, c, 512 * half : 512 * half + glen],
                                in_=ps[:, 0:glen],
                            )

                    if store_mode == "linear":
                        # timing-only: 4 stores x [128, 3748] covering the
                        # same output bytes with 14992 B linear descriptors
                        flat = outp[b].rearrange("w f -> (w f)")
                        n = 128 * 3736
                        for i in range(4):
                            nc.sync.dma_start(
                                out=flat[i * n : (i + 1) * n].rearrange(
                                    "(p q) -> p q", p=128, q=3736
                                ),
                                in_=t_t[:, :, :].rearrange("p c g -> p (c g)")[
                                    :, 0:3736
                                ],
                            )
                        continue
                    for h in cfg.get("store_hs", range(H)):
                        # DRAM rows 512*h + c*128 + p; descriptors are
                        # contiguous 3736 B f-runs either way.
                        if store_mode == "per_c":
                            # one store per c-block: [128, 934], DRAM fully
                            # sequential within the store
                            for c in range(4):
                                nc.sync.dma_start(
                                    out=outp[
                                        b,
                                        512 * h + 128 * c : 512 * h + 128 * (c + 1),
                                        :,
                                    ],
                                    in_=t_t[:, c, h : h + F],
                                )
                            continue
                        dram = outp[b, 512 * h : 512 * (h + 1), :].rearrange(
                            "(c p) f -> p c f", c=4, p=128
                        )
                        if split_io:
                            fsplit = 512 - h
                            nc.sync.dma_start(
                                out=dram[:, :, 0:fsplit],
                                in_=t_t[:, :, h : h + fsplit],
                            )
                            nc.sync.dma_start(
                                out=dram[:, :, fsplit:F],
                                in_=t_t[:, :, 512 : h + F],
                            )
                        else:
                            eng = (
                                nc.scalar
                                if (b * H + h) in act_slots
                                else nc.sync
                            )
                            eng.dma_start(
                                out=dram, in_=t_t[:, :, h : h + F]
                            )

    # TRN2 Matmult (and most instructions) encode at most 1 sync wait; the
    # Tile flow skips the bacc pass that splits extra waits into
    # InstEventSemaphore carriers, so run it here.
    import bass_rust

    bass_rust.generate_event_semaphores(nc)
    return nc


class _Runner:
    """Persistent jitted SPMD runner (modeled on bass2jax.run_bass_via_pjrt,
    but caches the jitted executable across calls).

    donate=False keeps the zero output-donor buffers reusable across calls,
    which lets timing loops run with fully device-resident operands."""

    def __init__(self, reps: int, donate: bool = True, variant: str = "v1L"):
        import jax
        from concourse import bass2jax, mybir
        from jax.experimental.shard_map import shard_map
        from jax.sharding import Mesh, PartitionSpec

        bass2jax.install_neuronx_cc_hook()
        self._jax = jax
        nc = _build_program(reps, variant)
        self._nc = nc

        partition_name = (
            nc.partition_id_tensor.name if nc.partition_id_tensor else None
        )
        in_names: list[str] = []
        out_names: list[str] = []
        out_avals = []
        self._zero_shapes = []
        for alloc in nc.m.functions[0].allocations:
            if not isinstance(alloc, mybir.MemoryLocationSet):
                continue
            name = alloc.memorylocations[0].name
            if alloc.kind == "ExternalInput":
                if name != partition_name:
                    in_names.append(name)
            elif alloc.kind == "ExternalOutput":
                out_names.append(name)
                shape = tuple(alloc.tensor_shape)
                dtype = mybir.dt.np(alloc.dtype)
                out_avals.append(jax.core.ShapedArray(shape, dtype))
                self._zero_shapes.append((shape, dtype))
        n_params = len(in_names)
        n_outs = len(out_avals)
        in_names_full = [*in_names, *out_names]
        if partition_name is not None:
            in_names_full.append(partition_name)

        def _body(*args):
            operands = list(args)
            if partition_name is not None:
                operands.append(bass2jax.partition_id_tensor())
            outs = bass2jax._bass_exec_p.bind(
                *operands,
                out_avals=tuple(out_avals),
                in_names=tuple(in_names_full),
                out_names=tuple(out_names),
                lowering_input_output_aliases=(),
                sim_require_finite=True,
                sim_require_nnan=True,
                nc=nc,
            )
            return tuple(outs)

        devices = jax.devices()[:N_CORES]
        assert len(devices) == N_CORES, devices
        mesh = Mesh(np.asarray(devices), ("core",))
        self._mesh = mesh
        self._pspec = PartitionSpec("core")
        donate_argnums = (
            tuple(range(n_params, n_params + n_outs)) if donate else ()
        )
        self._sharded = jax.jit(
            shard_map(
                _body,
                mesh=mesh,
                in_specs=(PartitionSpec("core"),) * (n_params + n_outs),
                out_specs=(PartitionSpec("core"),) * n_outs,
                check_rep=False,
            ),
            donate_argnums=donate_argnums,
            keep_unused=True,
        )

    def fresh_zeros(self):
        return [
            np.zeros((N_CORES * s[0], *s[1:]), d) for s, d in self._zero_shapes
        ]

    def __call__(self, x: np.ndarray, zeros=None):
        # shard_map splits axis 0 across the 8 cores: rows [2i, 2i+2) land on
        # core i — exactly the batch sharding. Global in/out pass through.
        if zeros is None:
            zeros = self.fresh_zeros()
        out = self._sharded(x, *zeros)[0]
        return np.asarray(out)

    def device_args(self, x: np.ndarray):
        """device_put the operands once, sharded over the mesh."""
        import jax
        from jax.sharding import NamedSharding

        sh = NamedSharding(self._mesh, self._pspec)
        return [jax.device_put(a, sh) for a in (x, *self.fresh_zeros())]

    def dispatch(self, args):
        """Launch without fetching results; returns device array handles."""
        return self._sharded(*args)


def get_runner(reps: int = 1, donate: bool = True, variant: str = "v1L") -> "_Runner":
    key = ("runner", reps, donate, variant)
    if key not in _CACHE:
        _CACHE[key] = _Runner(reps, donate, variant)
    return _CACHE[key]


def kernel(input: np.ndarray) -> np.ndarray:
    x = np.ascontiguousarray(input, dtype=np.float32)
    assert x.shape == (BATCH, S), x.shape
    return get_runner(1)(x)


# revision 54
# speedup vs baseline: 1.0617x; 1.0617x over previous
"""Enframe kernel for Trainium2 (Bass/Tile), 8-core data parallel.

Problem: input (16, 480000) f32, frame_length=2048, hop=512.
  out[b, w, f] = input[b, w + 512*f],  f in [0, 934), w in [0, 2048).

Key identity: write w = 512*h + l (h in [0,4), l in [0,512)). Then
  out[b, 512*h + l, f] = input[b, 512*(f + h) + l] = in3[b, f + h, l]
where in3 = input[:, :937*512].reshape(B, 937, 512). So the whole op is ONE
(937, 512) -> (512, 937) transpose per clip; the four h-blocks of the output
are shifted overlapping windows T[:, h : h+934] of that transpose.

Shipped default "v1L" (~64-68 us/iter measured, DMA-bound; the v1 family
ran crash-free over hundreds of HW executions):
  - load in3 rows as SBUF A[p = g%128, g//128, 512] (contiguous 2 KB rows),
    split at the h8=4 boundary so the first transposes start after ~1 MB
  - 32 TensorE 128x128 transposes per clip (f32 via identity) into PSUM,
    DVE-copy to SBUF T[p = l%128, l//128, g]
  - 4 stores per clip: DRAM rows (c p) <- T[:, :, h:h+934] via permuted DRAM
    AP; every DMA descriptor is a contiguous 3736 B run.

The "v8*" variants reach ~57-60 us (the measured pure-DMA ceiling) via an
interleaved partition mapping (out row l = 4q + j on partition q, per-h
[128, 4, 934] T tiles -> 14944 B store descriptors), but that family showed
sporadic NRT_EXEC_UNIT_UNRECOVERABLE crashes (4 across ~30 fresh processes,
on both ACT- and DVE-permute versions), so it is not the default. The v1
family ran crash-free for hundreds of executions all session. Known-fatal
on this HW: strided-free-dim f32 lhsT in a transpose matmul (crashes the
NC deterministically).
"""

import numpy as np

N_CORES = 8
BATCH = 16
B = BATCH // N_CORES  # clips per core
S = 480000
FRAME = 2048
HOP = 512
F = (S - FRAME) // HOP + 1  # 934
G = FRAME // HOP + F - 1  # 937 distinct 512-sample rows used
G_FULL = G // 128  # 7 full partition chunks
G_TAIL = G - 128 * G_FULL  # 41
H = FRAME // HOP  # 4 output row-blocks of 512

_CACHE: dict = {}


_VARIANTS = {
    # store_mode: "merged" (4 stores/clip, 1.9 MB, p-major enumeration) or
    #             "per_c" (16 stores/clip, 478 KB, sequential DRAM)
    # split_io: cut loads/stores at the psum-half boundary for earlier starts
    "v1": dict(store_mode="merged", split_io=False, bufs=2, psum_bufs=4),
    "v1p": dict(store_mode="merged", split_io=False, bufs=2, psum_bufs=8),
    # split only the loads (not stores): earlier transpose start, same stores
    "v1L": dict(store_mode="merged", split_io=False, split_loads=True, bufs=2, psum_bufs=4),
    # v1L with a 3rd T buffer: decouple copies from store-slot release
    "v1Lt": dict(store_mode="merged", split_io=False, split_loads=True, bufs=2, t_bufs=3, psum_bufs=4),
    "v2": dict(store_mode="merged", split_io=True, bufs=2, psum_bufs=8),
    "v3": dict(store_mode="per_c", split_io=False, bufs=2, psum_bufs=4),
    "v4": dict(store_mode="merged", split_io=False, bufs=3, psum_bufs=8),
    "v5": dict(store_mode="per_c", split_io=False, bufs=3, psum_bufs=8),
    # ring balance: n of the 8 stores go to the ACT (scalar) ring alongside
    # the loads, to even out bytes between the two HWDGE rings
    "v6": dict(
        store_mode="merged", split_io=False, bufs=2, psum_bufs=4, act_stores=3
    ),
    "v7": dict(
        store_mode="merged", split_io=False, bufs=2, psum_bufs=4, act_stores=2
    ),
    # timing-only: same DMAs, no transpose/copies — measures the pure DMA
    # ceiling of this access pattern (output is garbage)
    "dma": dict(
        store_mode="merged", split_io=False, bufs=2, psum_bufs=4, dma_only=True
    ),
    # dma-only with only half the stores: separates bytes-bound from
    # overhead-bound
    "dma2": dict(
        store_mode="merged",
        split_io=False,
        bufs=2,
        psum_bufs=4,
        dma_only=True,
        store_hs=(0, 1),
    ),
    # dma-only, same bytes but idealized stores: 14992 B descriptors into
    # fully linear DRAM — probes whether descriptor size lifts write BW
    "dma3": dict(
        store_mode="linear", split_io=False, bufs=2, psum_bufs=4, dma_only=True
    ),
    # interleaved partition mapping: output row l = 4q + j lives on partition
    # q, T tiles are per-h [128, 4, 934] so (j, f) merge into one contiguous
    # 3736-element run -> real 14944 B store descriptors
    # final: interleaved partition mapping with contiguous lhsT via ACT
    # pre-permute. NOTE: adding act_stores or split_io here caused
    # NRT_EXEC_UNIT_UNRECOVERABLE crashes (as "v9") — do not re-add.
    "v8": dict(store_mode="interleaved", split_io=False, bufs=2, psum_bufs=4),
    "v8p": dict(store_mode="interleaved", split_io=False, bufs=2, psum_bufs=8),
    # like v8p but the column pre-permute runs on DVE instead of ACT — the
    # ACT-copy version crashed sporadically (NRT_EXEC_UNIT_UNRECOVERABLE)
    "v8d": dict(
        store_mode="interleaved",
        split_io=False,
        bufs=2,
        psum_bufs=8,
        dve_permute=True,
    ),
}


def _build_program(reps: int, variant: str = "v1Lt"):
    from concourse import bass, masks, mybir
    from concourse.tile import TileContext

    cfg = _VARIANTS[variant]
    split_io = cfg["split_io"]
    store_mode = cfg["store_mode"]
    bufs = cfg["bufs"]
    psum_bufs = cfg["psum_bufs"]
    act_stores = cfg.get("act_stores", 0)
    # spread the ACT-ring stores evenly over the 8 (b, h) store slots
    act_slots = set()
    if act_stores:
        stride = (B * H) / act_stores
        act_slots = {int(i * stride + stride / 2) for i in range(act_stores)}

    F32 = mybir.dt.float32
    nc = bass.Bass()
    inp = nc.declare_dram_parameter("input", [B, S], F32, isOutput=False)
    outp = nc.declare_dram_parameter("out", [B, FRAME, F], F32, isOutput=True)

    with TileContext(nc) as tc:
        with (
            tc.tile_pool(name="ident_pool", bufs=1) as ipool,
            tc.tile_pool(name="a_pool", bufs=bufs) as apool,
            tc.tile_pool(name="t_pool", bufs=cfg.get("t_bufs", bufs)) as tpool,
            tc.tile_pool(name="psum_pool", bufs=psum_bufs, space="PSUM") as ppool,
        ):
            ident = ipool.tile([128, 128], F32)
            masks.make_identity(nc, ident[:])

            for _rep in range(reps):
                # loads for both clips upfront (own HWDGE ring via nc.scalar):
                # split at the h8=4 boundary so half-0 transposes start after
                # the first MB.
                a_ts = []
                for b in range(B):
                    a_t = apool.tile([128, G_FULL + 1, HOP], F32, tag="a")
                    a_ts.append(a_t)
                    # rows g = h8*128 + p hold samples 512g .. 512g+512
                    if split_io or cfg.get("split_loads"):
                        nc.scalar.dma_start(
                            out=a_t[:, 0:4, :],
                            in_=inp[b, 0 : 128 * 4 * HOP].rearrange(
                                "(h p c) -> p h c", h=4, p=128, c=HOP
                            ),
                        )
                        nc.scalar.dma_start(
                            out=a_t[:, 4:G_FULL, :],
                            in_=inp[
                                b, 128 * 4 * HOP : 128 * G_FULL * HOP
                            ].rearrange(
                                "(h p c) -> p h c", h=G_FULL - 4, p=128, c=HOP
                            ),
                        )
                    else:
                        nc.scalar.dma_start(
                            out=a_t[:, 0:G_FULL, :],
                            in_=inp[b, 0 : 128 * G_FULL * HOP].rearrange(
                                "(h p c) -> p h c", h=G_FULL, p=128, c=HOP
                            ),
                        )
                    # tail: last 41 rows
                    nc.scalar.dma_start(
                        out=a_t[0:G_TAIL, G_FULL, :],
                        in_=inp[b, 128 * G_FULL * HOP : G * HOP].rearrange(
                            "(p c) -> p c", p=G_TAIL, c=HOP
                        ),
                    )

                for b in range(B):
                    a_t = a_ts[b]
                    if store_mode == "interleaved":
                        # T2h[q, j, f] = out[b, 512h + 4q + j, f]; per-h tiles
                        # of exactly [128, 4, 934] make (j, f) contiguous per
                        # partition -> 14944 B store descriptors.
                        #
                        # A strided-free-dim f32 lhsT crashes the NC
                        # (NRT_EXEC_UNIT_UNRECOVERABLE, probed in isolation),
                        # so pre-permute columns on ACT: a_perm[p, h8, j, q] =
                        # a_t[p, h8, 4q + j]; every matmul then reads a
                        # contiguous 128-column slice.
                        a_perm = apool.tile(
                            [128, G_FULL + 1, 4, 128], F32, tag="a_perm"
                        )
                        perm_copy = (
                            nc.vector.tensor_copy
                            if cfg.get("dve_permute")
                            else nc.scalar.copy
                        )
                        perm_copy(
                            out=a_perm[:, 0:G_FULL, :, :],
                            in_=a_t[:, 0:G_FULL, :].rearrange(
                                "p h (q j) -> p h j q", q=128, j=4
                            ),
                        )
                        perm_copy(
                            out=a_perm[0:G_TAIL, G_FULL, :, :],
                            in_=a_t[0:G_TAIL, G_FULL, :].rearrange(
                                "p (q j) -> p j q", q=128, j=4
                            ),
                        )
                        t2 = [
                            tpool.tile(
                                [128, 4, F], F32, tag=f"t2_{h}", name=f"t2_{h}"
                            )
                            for h in range(H)
                        ]
                        for j in range(4):
                            for half in range(2):
                                ps = ppool.tile([128, 512], F32, tag="ps")
                                glen = 512 if half == 0 else G - 512  # 425
                                for k in range(4):
                                    h8 = 4 * half + k
                                    rows = 128 if h8 < G_FULL else G_TAIL
                                    nc.tensor.transpose(
                                        out=ps[:, 128 * k : 128 * k + rows],
                                        in_=a_perm[0:rows, h8, j, :],
                                        identity=ident[0:rows, 0:rows],
                                    )
                                # ps[q, col] = T row (4q+j), g = 512*half+col
                                for h in range(H):
                                    if half == 0:
                                        # f in [0, 512-h) <- g = h + f
                                        nc.vector.tensor_copy(
                                            out=t2[h][:, j, 0 : 512 - h],
                                            in_=ps[:, h:512],
                                        )
                                    else:
                                        # f in [512-h, ...) <- g = h + f
                                        ln = min(422 + h, glen)
                                        nc.vector.tensor_copy(
                                            out=t2[h][:, j, 512 - h : 512 - h + ln],
                                            in_=ps[:, 0:ln],
                                        )
                        for h in range(H):
                            eng = (
                                nc.scalar
                                if (b * H + h) in act_slots
                                else nc.sync
                            )
                            eng.dma_start(
                                out=outp[b, 512 * h : 512 * (h + 1), :].rearrange(
                                    "(q j) f -> q (j f)", q=128, j=4
                                ),
                                in_=t2[h][:, :, :].rearrange("p j f -> p (j f)"),
                            )
                        continue
                    t_t = tpool.tile([128, 4, G], F32, tag="t")
                    if cfg.get("dma_only"):
                        # give t_t a writer so Tile allocates it
                        nc.vector.memset(t_t[:, 0, 0:1], 0.0)
                    for c in range(4):
                        if cfg.get("dma_only"):
                            break
                        for half in range(2):
                            ps = ppool.tile([128, 512], F32, tag="ps")
                            glen = 512 if half == 0 else G - 512  # 425
                            for k in range(4):
                                h8 = 4 * half + k
                                rows = 128 if h8 < G_FULL else G_TAIL
                                nc.tensor.transpose(
                                    out=ps[:, 128 * k : 128 * k + rows],
                                    in_=a_t[0:rows, h8, 128 * c : 128 * (c + 1)],
                                    identity=ident[0:rows, 0:rows],
                                )
                            nc.vector.tensor_copy(
                                out=t_t[:, c, 512 * half : 512 * half + glen],
                                in_=ps[:, 0:glen],
                            )

                    if store_mode == "linear":
                        # timing-only: 4 stores x [128, 3748] covering the
                        # same output bytes with 14992 B linear descriptors
                        flat = outp[b].rearrange("w f -> (w f)")
                        n = 128 * 3736
                        for i in range(4):
                            nc.sync.dma_start(
                                out=flat[i * n : (i + 1) * n].rearrange(
                                    "(p q) -> p q", p=128, q=3736
                                ),
                                in_=t_t[:, :, :].rearrange("p c g -> p (c g)")[
                                    :, 0:3736
                                ],
                            )
                        continue
                    for h in cfg.get("store_hs", range(H)):
                        # DRAM rows 512*h + c*128 + p; descriptors are
                        # contiguous 3736 B f-runs either way.
                        if store_mode == "per_c":
                            # one store per c-block: [128, 934], DRAM fully
                            # sequential within the store
                            for c in range(4):
                                nc.sync.dma_start(
                                    out=outp[
                                        b,
                                        512 * h + 128 * c : 512 * h + 128 * (c + 1),
                                        :,
                                    ],
                                    in_=t_t[:, c, h : h + F],
                                )
                            continue
                        dram = outp[b, 512 * h : 512 * (h + 1), :].rearrange(
                            "(c p) f -> p c f", c=4, p=128
                        )
                        if split_io:
                            fsplit = 512 - h
                            nc.sync.dma_start(
                                out=dram[:, :, 0:fsplit],
                                in_=t_t[:, :, h : h + fsplit],
                            )
                            nc.sync.dma_start(
                                out=dram[:, :, fsplit:F],
                                in_=t_t[:, :, 512 : h + F],
                            )
                        else:
                            eng = (
                                nc.scalar
                                if (b * H + h) in act_slots
                                else nc.sync
                            )
                            eng.dma_start(
                                out=dram, in_=t_t[:, :, h : h + F]
                            )

    # TRN2 Matmult (and most instructions) encode at most 1 sync wait; the
    # Tile flow skips the bacc pass that splits extra waits into
    # InstEventSemaphore carriers, so run it here.
    import bass_rust

    bass_rust.generate_event_semaphores(nc)
    return nc


class _Runner:
    """Persistent jitted SPMD runner (modeled on bass2jax.run_bass_via_pjrt,
    but caches the jitted executable across calls).

    donate=False keeps the zero output-donor buffers reusable across calls,
    which lets timing loops run with fully device-resident operands."""

    def __init__(self, reps: int, donate: bool = True, variant: str = "v1Lt"):
        import jax
        from concourse import bass2jax, mybir
        from jax.experimental.shard_map import shard_map
        from jax.sharding import Mesh, PartitionSpec

        bass2jax.install_neuronx_cc_hook()
        self._jax = jax
        nc = _build_program(reps, variant)
        self._nc = nc

        partition_name = (
            nc.partition_id_tensor.name if nc.partition_id_tensor else None
        )
        in_names: list[str] = []
        out_names: list[str] = []
        out_avals = []
        self._zero_shapes = []
        for alloc in nc.m.functions[0].allocations:
            if not isinstance(alloc, mybir.MemoryLocationSet):
                continue
            name = alloc.memorylocations[0].name
            if alloc.kind == "ExternalInput":
                if name != partition_name:
                    in_names.append(name)
            elif alloc.kind == "ExternalOutput":
                out_names.append(name)
                shape = tuple(alloc.tensor_shape)
                dtype = mybir.dt.np(alloc.dtype)
                out_avals.append(jax.core.ShapedArray(shape, dtype))
                self._zero_shapes.append((shape, dtype))
        n_params = len(in_names)
        n_outs = len(out_avals)
        in_names_full = [*in_names, *out_names]
        if partition_name is not None:
            in_names_full.append(partition_name)

        def _body(*args):
            operands = list(args)
            if partition_name is not None:
                operands.append(bass2jax.partition_id_tensor())
            outs = bass2jax._bass_exec_p.bind(
                *operands,
                out_avals=tuple(out_avals),
                in_names=tuple(in_names_full),
                out_names=tuple(out_names),
                lowering_input_output_aliases=(),
                sim_require_finite=True,
                sim_require_nnan=True,
                nc=nc,
            )
            return tuple(outs)

        devices = jax.devices()[:N_CORES]
        assert len(devices) == N_CORES, devices
        mesh = Mesh(np.asarray(devices), ("core",))
        self._mesh = mesh
        self._pspec = PartitionSpec("core")
        donate_argnums = (
            tuple(range(n_params, n_params + n_outs)) if donate else ()
        )
        self._sharded = jax.jit(
            shard_map(
                _body,
                mesh=mesh,
                in_specs=(PartitionSpec("core"),) * (n_params + n_outs),
                out_specs=(PartitionSpec("core"),) * n_outs,
                check_rep=False,
            ),
            donate_argnums=donate_argnums,
            keep_unused=True,
        )

    def fresh_zeros(self):
        return [
            np.zeros((N_CORES * s[0], *s[1:]), d) for s, d in self._zero_shapes
        ]

    def __call__(self, x: np.ndarray, zeros=None):
        # shard_map splits axis 0 across the 8 cores: rows [2i, 2i+2) land on
        # core i — exactly the batch sharding. Global in/out pass through.
        if zeros is None:
            zeros = self.fresh_zeros()
        out = self._sharded(x, *zeros)[0]
        return np.asarray(out)

    def device_args(self, x: np.ndarray):
        """device_put the operands once, sharded over the mesh."""
        import jax
        from jax.sharding import NamedSharding

        sh = NamedSharding(self._mesh, self._pspec)
        return [jax.device_put(a, sh) for a in (x, *self.fresh_zeros())]

    def dispatch(self, args):
        """Launch without fetching results; returns device array handles."""
        return self._sharded(*args)


def get_runner(reps: int = 1, donate: bool = True, variant: str = "v1Lt") -> "_Runner":
    key = ("runner", reps, donate, variant)
    if key not in _CACHE:
        _CACHE[key] = _Runner(reps, donate, variant)
    return _CACHE[key]


def kernel(input: np.ndarray) -> np.ndarray:
    x = np.ascontiguousarray(input, dtype=np.float32)
    assert x.shape == (BATCH, S), x.shape
    return get_runner(1)(x)
